# revision 9
# baseline (speedup 1.0000x reference)
"""Trainium2 Bass kernel for nn_CDRsAttention (sparse multi-head attention
with padding mask + CDR key mask on the first 2 heads).

Sharding: 8 cores = 4 samples (B) x 2 head groups. Core (b, g) computes
heads [g, g+2, g+4, g+6] of sample b (exactly one CDR head each), producing
a partial output ctx_heads @ Wo_rows; the host sums the two partials + bo.

Host-side prep (pure numpy, cheap):
  - per-sample key gather: only keys with mask==1 participate, CDR-valid
    keys first, then regular keys, zero-padded to NK = 128*ceil(max valid).
    No inter-region padding: the CDR head attends tiles [0, NKT_CDR) and
    masks intruding regular keys via a per-partition additive bias (-30)
    fed to the exp activation for the boundary tiles.
  - xkT carries one extra row (valid-key indicator) that flows through
    augmented Wv selector columns so v column h*65+64 is the indicator row,
    making ctx^T row 64 the softmax denominator (padded keys drop out).
  - q/k biases are zero, so their projections contract over exactly 512
    rows (no augmented row).

Device per core (bf16 matmuls):
  qT/kT/v projections -> per head: S^T = kT_tile^T @ qT (keys on psum
  partitions), P = exp(S^T/8) on ScalarE straight out of PSUM (pairs of
  ktiles per activate), ctx^T accumulated as v_aug^T @ P, then a
  fast-reciprocal of the denominator row, a PE broadcast matmul (f32r),
  a DVE normalization multiply, and the output projection
  out = ctx_norm^T.T @ Wo_rows streamed straight to DRAM per 128-query
  slab. q/out projections are interleaved as PE filler between attention
  groups to keep the PE array dense and HAM-warm.
"""
import math
from contextlib import ExitStack

import ml_dtypes
import numpy as np

import concourse.bass as bass
import concourse.mybir as mybir
import concourse.tile as tile
from concourse import bacc
from concourse.bass_utils import run_bass_kernel_spmd

B, T, C, H, D = 4, 2048, 512, 8, 64
F32 = mybir.dt.float32
F32R = mybir.dt.float32r
BF16 = mybir.dt.bfloat16
EXP_SCALE = 1.0 / 8.0  # 1/sqrt(D)
MASK_BIAS = -30.0

_PROGRAM_CACHE: dict = {}
LAST_RESULTS = None  # BassKernelResults of the most recent kernel() call


def _chunks(total, step):
    return [(i, min(step, total - i)) for i in range(0, total, step)]


def _build_program(NKT, NKT_CDR, BIAS0, NBT):
    NK = NKT * 128
    nc = bacc.Bacc("TRN2", target_bir_lowering=False, debug=False, num_devices=8)
    xT_d = nc.dram_tensor("xT", [C, T], BF16, kind="ExternalInput").ap()
    xkT_d = nc.dram_tensor("xkT", [C + 1, NK], BF16, kind="ExternalInput").ap()
    wq_d = nc.dram_tensor("Wq", [C, 256], BF16, kind="ExternalInput").ap()
    wk_d = nc.dram_tensor("Wk", [C, 256], BF16, kind="ExternalInput").ap()
    wv_d = nc.dram_tensor("Wv", [C + 1, 260], BF16, kind="ExternalInput").ap()
    wo_d = nc.dram_tensor("Wo", [256, 512], BF16, kind="ExternalInput").ap()
    cdrb_d = nc.dram_tensor("cdrb", [128, max(NBT, 1)], F32,
                            kind="ExternalInput").ap()
    out_d = nc.dram_tensor("out", [T, 512], F32, kind="ExternalOutput").ap()

    with tile.TileContext(nc) as tc:
        with ExitStack() as ctx:
            _body(ctx, tc, xT_d, xkT_d, wq_d, wk_d, wv_d, wo_d, cdrb_d, out_d,
                  NK, NKT, NKT_CDR, BIAS0, NBT)
    nc.compile()
    return nc


def _body(ctx, tc, xT_d, xkT_d, wq_d, wk_d, wv_d, wo_d, cdrb_d, out_d,
          NK, NKT, NKT_CDR, BIAS0, NBT):
    nc = tc.nc
    Exp = mybir.ActivationFunctionType.Exp

    wpool = ctx.enter_context(tc.tile_pool(name="w", bufs=1))
    xpool = ctx.enter_context(tc.tile_pool(name="x", bufs=1))
    qkv = ctx.enter_context(tc.tile_pool(name="qkv", bufs=1))
    psS = ctx.enter_context(tc.tile_pool(name="psS", bufs=2, space="PSUM"))
    psC = ctx.enter_context(tc.tile_pool(name="psC", bufs=2, space="PSUM"))
    psM = ctx.enter_context(tc.tile_pool(name="psM", bufs=2, space="PSUM"))
    pP = ctx.enter_context(tc.tile_pool(name="pP", bufs=2))
    pR = ctx.enter_context(tc.tile_pool(name="pR", bufs=2))
    pO = ctx.enter_context(tc.tile_pool(name="pO", bufs=3))

    # ---- input loads: sync queue feeds k/v path, gpsimd queue the rest ----
    def load4(pool, dram, cols, nm, eng, col_chunks=None):
        """[512, cols] DRAM -> [128, 4*cols] tile; 4 contraction-chunk views."""
        main = pool.tile([128, 4 * cols], BF16, name=f"{nm}m", tag=f"{nm}m")
        mv = main[:].rearrange("p (ch c) -> p ch c", ch=4)
        for n0, ns in (col_chunks or [(0, cols)]):
            eng.dma_start(
                mv[:, :, n0:n0 + ns],
                dram[0:C, n0:n0 + ns].rearrange("(ch p) c -> p ch c", p=128))
        return [main[:, ci * cols:(ci + 1) * cols] for ci in range(4)]

    wks = load4(wpool, wk_d, 256, "wk", nc.sync)
    xks = load4(xpool, xkT_d, NK, "xk", nc.sync, col_chunks=_chunks(NK, 512))
    wvs = load4(wpool, wv_d, 260, "wv", nc.sync)
    wqs = load4(wpool, wq_d, 256, "wq", nc.sync)

    xs = load4(xpool, xT_d, T, "x", nc.gpsimd, col_chunks=_chunks(T, 512))
    xk_aug = xpool.tile([1, NK], BF16, name="xka", tag="xka")
    nc.gpsimd.dma_start(xk_aug[:], xkT_d[C:C + 1, :])
    wv_aug = wpool.tile([1, 260], BF16, name="wva", tag="wva")
    nc.gpsimd.dma_start(wv_aug[:], wv_d[C:C + 1, :])
    cdrb = wpool.tile([128, max(NBT, 1)], F32, name="cdrb", tag="cdrb")
    nc.gpsimd.dma_start(cdrb[:], cdrb_d[:])
    wo_all = wpool.tile([128, 1024], BF16, name="wo", tag="wo")
    nc.gpsimd.dma_start(wo_all[:].rearrange("p (g c) -> p g c", g=2),
                        wo_d[:].rearrange("(g p) c -> p g c", p=128))
    wo = [wo_all[:, 0:512], wo_all[:, 512:1024]]

    # selector matrix for denominator broadcast: E.T @ cu = row 64 of cu
    # replicated across 64 psum partitions
    esel = wpool.tile([65, 64], BF16, name="esel", tag="esel")
    nc.vector.memset(esel[:], 0.0)
    nc.vector.memset(esel[64:65, :], 1.0)

    # ---- persistent activation tiles ------------------------------------
    qT = [qkv.tile([128, T], BF16, name=f"q{p}", tag=f"q{p}") for p in range(2)]
    kT = [qkv.tile([128, NK], BF16, name=f"k{p}", tag=f"k{p}") for p in range(2)]
    v_sb = qkv.tile([128, NKT * 260], BF16, name="v", tag="v")
    ctxn = [qkv.tile([128, T], BF16, name=f"ctxn{p}", tag=f"ctxn{p}")
            for p in range(2)]

    # ---- projection emitters --------------------------------------------
    def k_proj(pp):
        for n0, ns in _chunks(NK, 512):
            mt = psM.tile([128, 512], F32, name="m", tag="m")
            for ci in range(4):
                nc.tensor.matmul(
                    mt[:, :ns],
                    wks[ci][:, pp * 128:(pp + 1) * 128],
                    xks[ci][:, n0:n0 + ns],
                    start=(ci == 0), stop=(ci == 3))
            nc.vector.tensor_copy(kT[pp][:, n0:n0 + ns], mt[:, :ns])

    def v_proj(kt):
        mt = psM.tile([128, 512], F32, name="m", tag="m")
        for ci in range(4):
            nc.tensor.matmul(
                mt[:, 0:260],
                xks[ci][:, kt * 128:(kt + 1) * 128],
                wvs[ci][:],
                start=(ci == 0), stop=False)
        nc.tensor.matmul(
            mt[:, 0:260],
            xk_aug[0:1, kt * 128:(kt + 1) * 128],
            wv_aug[:],
            start=False, stop=True)
        nc.vector.tensor_copy(v_sb[:, kt * 260:(kt + 1) * 260], mt[:, 0:260])

    def q_proj(qc, pp):
        mt = psM.tile([128, 512], F32, name="m", tag="m")
        for ci in range(4):
            nc.tensor.matmul(
                mt[:],
                wqs[ci][:, pp * 128:(pp + 1) * 128],
                xs[ci][:, qc * 512:(qc + 1) * 512],
                start=(ci == 0), stop=(ci == 3))
        nc.vector.tensor_copy(qT[pp][:, qc * 512:(qc + 1) * 512], mt[:])

    def out_proj(qc, tqs):
        for tq in tqs:
            t0 = qc * 512 + tq * 128
            cp = psM.tile([128, 512], F32, name="m", tag="m")
            nc.tensor.matmul(cp[:], ctxn[0][:, t0:t0 + 128], wo[0],
                             start=True, stop=False)
            nc.tensor.matmul(cp[:], ctxn[1][:, t0:t0 + 128], wo[1],
                             start=False, stop=True)
            ot = pO.tile([128, 512], F32, name="ot", tag="o")
            nc.vector.tensor_copy(ot[:], cp[:])
            nc.sync.dma_start(out_d[t0:t0 + 128, :], ot[:])

    # ---- attention ------------------------------------------------------
    def attention(qc, pair):
        """Generator: yields once per ktile super-group so the caller can
        ration filler matmuls into the exp-wait bubbles."""
        q0 = qc * 512
        nts = [NKT_CDR if pair == 0 else NKT, NKT]
        Pt = [pP.tile([128, NKT * 512], BF16, name=f"P{h}", tag="P")
              for h in range(2)]
        cp = [psC.tile([65, 512], F32, name=f"c{h}", tag="ctx")
              for h in range(2)]
        done = [0, 0]
        for g0 in range(0, NKT, 2):
            kts = {h: [kt for kt in (g0, g0 + 1) if kt < nts[h]]
                   for h in range(2)}
            sp = {h: psS.tile([128, 1024], F32, name=f"S{h}", tag="S")
                  for h in range(2) if kts[h]}
            # score matmuls interleaved across heads: adjacent instructions
            # target distinct PE row groups and run concurrently
            for j in range(2):
                for h in range(2):
                    if j < len(kts[h]):
                        kt = kts[h][j]
                        r0 = h * 64
                        nc.tensor.matmul(
                            sp[h][:, j * 512:(j + 1) * 512],
                            kT[pair][r0:r0 + 64, kt * 128:(kt + 1) * 128],
                            qT[pair][r0:r0 + 64, q0:q0 + 512],
                            start=True, stop=True,
                            tile_position=(r0, 0))
            for h in range(2):
                if not kts[h]:
                    continue
                is_cdr = (pair == 0 and h == 0)
                if is_cdr and NBT > 0 and (g0 + len(kts[h]) > BIAS0):
                    for j, kt in enumerate(kts[h]):
                        bias = (cdrb[:, kt - BIAS0:kt - BIAS0 + 1]
                                if kt >= BIAS0 else 0.0)
                        nc.scalar.activation(
                            Pt[h][:, kt * 512:(kt + 1) * 512],
                            sp[h][:, j * 512:(j + 1) * 512],
                            Exp, bias=bias, scale=EXP_SCALE)
                else:
                    nc.scalar.activation(
                        Pt[h][:, g0 * 512:(g0 + len(kts[h])) * 512],
                        sp[h][:, 0:len(kts[h]) * 512],
                        Exp, scale=EXP_SCALE)
            for h in range(2):
                head = 2 * pair + h
                for kt in kts[h]:
                    nc.tensor.matmul(
                        cp[h][:],
                        v_sb[:, kt * 260 + head * 65:kt * 260 + (head + 1) * 65],
                        Pt[h][:, kt * 512:(kt + 1) * 512],
                        start=(done[h] == 0), stop=(done[h] + 1 == nts[h]))
                    done[h] += 1
                if done[h] == nts[h]:
                    # normalize: PE-broadcast the denominator row, then DVE
                    # fast-reciprocal + multiply at 64-partition width
                    r0 = h * 64
                    cu = pR.tile([65, 512], BF16, name="cu", tag="cu")
                    nc.vector.tensor_copy(cu[:], cp[h][:])
                    dn = psM.tile([128, 512], F32, name="m", tag="m")
                    nc.tensor.matmul(dn[0:64, :], esel[:], cu[:],
                                     start=True, stop=True)
                    rb = pR.tile([64, 512], F32, name="rb", tag="rb")
                    nc.vector.reciprocal_approx_fast(rb[:], dn[0:64, :])
                    nc.vector.tensor_mul(
                        ctxn[pair][r0:r0 + 64, q0:q0 + 512],
                        cu[0:64, :], rb[:])
                    done[h] += 1  # emit normalize once
            yield

    # ---- emission order -------------------------------------------------
    # PE warmup: dummy matmuls warm the HAM clock gate while input DMAs land
    wps = psM.tile([128, 512], F32, name="m", tag="m")
    for i in range(36):
        nc.tensor.matmul(wps[0:64, 0:64], esel[:], esel[:],
                         start=True, stop=True)
    nc.vector.tensor_copy(pR.tile([64, 64], F32, name="wd", tag="wd")[:],
                          wps[0:64, 0:64])

    k_proj(0)
    q_proj(0, 0)
    for kt in range(NKT):
        v_proj(kt)
    k_proj(1)
    q_proj(0, 1)
    for qc in range(4):
        for pair in range(2):
            fillers = []
            if qc > 0:
                tqs = (0, 1) if pair == 0 else (2, 3)
                fillers.append(lambda t=tqs[0]: out_proj(qc - 1, (t,)))
                fillers.append(lambda t=tqs[1]: out_proj(qc - 1, (t,)))
            if qc < 3:
                fillers.append(lambda p=pair: q_proj(qc + 1, p))
            fi = 0
            for _ in attention(qc, pair):
                if fi < len(fillers):
                    fillers[fi]()
                    fi += 1
            while fi < len(fillers):
                fillers[fi]()
                fi += 1
    out_proj(3, range(4))


# ---------------------------------------------------------------------------
# host side
# ---------------------------------------------------------------------------

def _host_prep(x, mask, cdrs_score, Wq, bq, Wk, bk, Wv, bv, Wo, bo):
    x = np.ascontiguousarray(np.asarray(x, np.float32))
    mask = np.asarray(mask)
    cdrs = np.asarray(cdrs_score)
    Wq = np.asarray(Wq, np.float32)
    Wk = np.asarray(Wk, np.float32)
    Wv = np.asarray(Wv, np.float32)
    Wo = np.asarray(Wo, np.float32)
    bv = np.asarray(bv, np.float32)
    assert np.abs(np.asarray(bq)).max() < 1e-6, "nonzero bq unsupported"
    assert np.abs(np.asarray(bk)).max() < 1e-6, "nonzero bk unsupported"

    gathers = []
    for b in range(B):
        valid = mask[b] == 1
        cdrv = valid & (cdrs[b] == 1) if np.any(cdrs[b] == 1) else valid
        regv = valid & ~cdrv
        gathers.append((np.nonzero(cdrv)[0], np.nonzero(regv)[0]))
    ncdrs = [len(g[0]) for g in gathers]
    valids = [len(g[0]) + len(g[1]) for g in gathers]
    NKT = max(1, math.ceil(max(valids) / 128))
    NK = NKT * 128
    NKT_CDR = max(1, math.ceil(max(ncdrs) / 128))
    BIAS0 = min(ncdrs) // 128
    NBT = NKT_CDR - BIAS0

    # per-group weight bundles (shared across samples)
    wbund = []
    for g in range(2):
        heads = [g, g + 2, g + 4, g + 6]
        dims = np.concatenate([np.arange(h * D, (h + 1) * D) for h in heads])
        wq_c = Wq[:, dims]
        wk_c = Wk[:, dims]
        wv_cols = []
        for h in heads:
            hd = np.arange(h * D, (h + 1) * D)
            wv = np.concatenate([Wv[:, hd], bv[hd][None, :]], axis=0)
            sel = np.zeros((C + 1, 1), np.float32)
            sel[C, 0] = 1.0
            wv_cols.append(np.concatenate([wv, sel], axis=1))
        wv_aug = np.concatenate(wv_cols, axis=1)
        wo_rows = Wo[dims, :]
        wbund.append(tuple(
            np.ascontiguousarray(w.astype(ml_dtypes.bfloat16))
            for w in (wq_c, wk_c, wv_aug, wo_rows)))

    in_maps = []
    for b in range(B):
        idx_cdr, idx_reg = gathers[b]
        nv = len(idx_cdr) + len(idx_reg)
        xk = np.zeros((NK, C), np.float32)
        xk[:len(idx_cdr)] = x[b, idx_cdr]
        xk[len(idx_cdr):nv] = x[b, idx_reg]
        ones_row = np.zeros((1, NK), np.float32)
        ones_row[0, :nv] = 1.0
        xkT_aug = np.ascontiguousarray(
            np.concatenate([xk.T, ones_row], axis=0))
        xT_bf = np.ascontiguousarray(x[b].T.astype(ml_dtypes.bfloat16))
        xkT_bf = np.ascontiguousarray(xkT_aug.astype(ml_dtypes.bfloat16))
        cdrb = np.zeros((128, max(NBT, 1)), np.float32)
        for t in range(NBT):
            keys = (BIAS0 + t) * 128 + np.arange(128)
            cdrb[:, t] = np.where(keys < len(idx_cdr), 0.0, MASK_BIAS)
        for g in range(2):
            wq_c, wk_c, wv_aug, wo_rows = wbund[g]
            in_maps.append({
                "xT": xT_bf, "xkT": xkT_bf,
                "Wq": wq_c, "Wk": wk_c, "Wv": wv_aug, "Wo": wo_rows,
                "cdrb": cdrb,
            })
    return in_maps, NKT, NKT_CDR, BIAS0, NBT


def kernel(**inputs) -> np.ndarray:
    global LAST_RESULTS
    in_maps, NKT, NKT_CDR, BIAS0, NBT = _host_prep(**inputs)

    key = (NKT, NKT_CDR, BIAS0, NBT)
    nc = _PROGRAM_CACHE.get(key)
    if nc is None:
        nc = _build_program(NKT, NKT_CDR, BIAS0, NBT)
        _PROGRAM_CACHE[key] = nc

    res = run_bass_kernel_spmd(nc, in_maps, core_ids=list(range(8)))
    LAST_RESULTS = res

    bo = np.asarray(inputs["bo"], np.float32)
    out = np.empty((B, T, C), np.float32)
    for b in range(B):
        out[b] = res.results[2 * b]["out"] + res.results[2 * b + 1]["out"] + bo[None, :]
    return out


# revision 12
# speedup vs baseline: 1.0244x; 1.0244x over previous
"""Trainium2 Bass kernel for nn_CDRsAttention (sparse multi-head attention
with padding mask + CDR key mask on the first 2 heads).

Sharding: 8 cores = 4 samples (B) x 2 head groups. Core (b, g) computes
heads [g, g+2, g+4, g+6] of sample b (exactly one CDR head each), producing
a partial output ctx_heads @ Wo_rows; the host sums the two partials + bo.

Host-side prep (pure numpy, cheap):
  - per-sample key gather: only keys with mask==1 participate, CDR-valid
    keys first, then regular keys, zero-padded to NK = 128*ceil(max valid).
    No inter-region padding: the CDR head attends tiles [0, NKT_CDR) and
    masks intruding regular keys via a per-partition additive bias (-30)
    fed to the exp activation for the boundary tiles.
  - xkT carries one extra row (valid-key indicator) that flows through
    augmented Wv selector columns so v column h*65+64 is the indicator row,
    making ctx^T row 64 the softmax denominator (padded keys drop out).
  - q/k biases are zero, so their projections contract over exactly 512
    rows (no augmented row).

Device per core (bf16 matmuls):
  qT/kT/v projections -> per head: S^T = kT_tile^T @ qT (keys on psum
  partitions), P = exp(S^T/8) on ScalarE straight out of PSUM (pairs of
  ktiles per activate), ctx^T accumulated as v_aug^T @ P, then a
  fast-reciprocal of the denominator row, a PE broadcast matmul (f32r),
  a DVE normalization multiply, and the output projection
  out = ctx_norm^T.T @ Wo_rows streamed straight to DRAM per 128-query
  slab. q/out projections are interleaved as PE filler between attention
  groups to keep the PE array dense and HAM-warm.
"""
import math
from contextlib import ExitStack

import ml_dtypes
import numpy as np

import concourse.bass as bass
import concourse.mybir as mybir
import concourse.tile as tile
from concourse import bacc
from concourse.bass_utils import run_bass_kernel_spmd

B, T, C, H, D = 4, 2048, 512, 8, 64
F32 = mybir.dt.float32
F32R = mybir.dt.float32r
BF16 = mybir.dt.bfloat16
EXP_SCALE = 1.0 / 8.0  # 1/sqrt(D)
MASK_BIAS = -30.0

_PROGRAM_CACHE: dict = {}
LAST_RESULTS = None  # BassKernelResults of the most recent kernel() call


def _chunks(total, step):
    return [(i, min(step, total - i)) for i in range(0, total, step)]


def _build_program(NKT, NKT_CDR, BIAS0, NBT):
    NK = NKT * 128
    nc = bacc.Bacc("TRN2", target_bir_lowering=False, debug=False, num_devices=8)
    xT_d = nc.dram_tensor("xT", [C, T], BF16, kind="ExternalInput").ap()
    xkT_d = nc.dram_tensor("xkT", [C + 1, NK], BF16, kind="ExternalInput").ap()
    wq_d = nc.dram_tensor("Wq", [C, 256], BF16, kind="ExternalInput").ap()
    wk_d = nc.dram_tensor("Wk", [C, 256], BF16, kind="ExternalInput").ap()
    wv_d = nc.dram_tensor("Wv", [C + 1, 260], BF16, kind="ExternalInput").ap()
    wo_d = nc.dram_tensor("Wo", [256, 512], BF16, kind="ExternalInput").ap()
    cdrb_d = nc.dram_tensor("cdrb", [128, max(NBT, 1)], F32,
                            kind="ExternalInput").ap()
    out_d = nc.dram_tensor("out", [T, 512], F32, kind="ExternalOutput").ap()

    with tile.TileContext(nc) as tc:
        with ExitStack() as ctx:
            _body(ctx, tc, xT_d, xkT_d, wq_d, wk_d, wv_d, wo_d, cdrb_d, out_d,
                  NK, NKT, NKT_CDR, BIAS0, NBT)
    nc.compile()
    return nc


def _body(ctx, tc, xT_d, xkT_d, wq_d, wk_d, wv_d, wo_d, cdrb_d, out_d,
          NK, NKT, NKT_CDR, BIAS0, NBT):
    nc = tc.nc
    Exp = mybir.ActivationFunctionType.Exp

    wpool = ctx.enter_context(tc.tile_pool(name="w", bufs=1))
    xpool = ctx.enter_context(tc.tile_pool(name="x", bufs=1))
    qkv = ctx.enter_context(tc.tile_pool(name="qkv", bufs=1))
    psS = ctx.enter_context(tc.tile_pool(name="psS", bufs=2, space="PSUM"))
    psC = ctx.enter_context(tc.tile_pool(name="psC", bufs=2, space="PSUM"))
    psM = ctx.enter_context(tc.tile_pool(name="psM", bufs=2, space="PSUM"))
    pP = ctx.enter_context(tc.tile_pool(name="pP", bufs=2))
    pR = ctx.enter_context(tc.tile_pool(name="pR", bufs=2))
    pO = ctx.enter_context(tc.tile_pool(name="pO", bufs=3))

    # ---- input loads: sync queue feeds k/v path, gpsimd queue the rest ----
    def load4(pool, dram, cols, nm, eng, col_chunks=None):
        """[512, cols] DRAM -> [128, 4*cols] tile; 4 contraction-chunk views."""
        main = pool.tile([128, 4 * cols], BF16, name=f"{nm}m", tag=f"{nm}m")
        mv = main[:].rearrange("p (ch c) -> p ch c", ch=4)
        for n0, ns in (col_chunks or [(0, cols)]):
            eng.dma_start(
                mv[:, :, n0:n0 + ns],
                dram[0:C, n0:n0 + ns].rearrange("(ch p) c -> p ch c", p=128))
        return [main[:, ci * cols:(ci + 1) * cols] for ci in range(4)]

    wks = load4(wpool, wk_d, 256, "wk", nc.sync)
    nkch = _chunks(NK, 512)
    xks_tile = xpool.tile([128, 4 * NK], BF16, name="xkm", tag="xkm")
    xks_v = xks_tile[:].rearrange("p (ch c) -> p ch c", ch=4)
    n0, ns = nkch[0]
    nc.sync.dma_start(xks_v[:, :, n0:n0 + ns],
                      xkT_d[0:C, n0:n0 + ns].rearrange("(ch p) c -> p ch c", p=128))
    wvs = load4(wpool, wv_d, 260, "wv", nc.sync)
    for n0, ns in nkch[1:]:
        nc.sync.dma_start(xks_v[:, :, n0:n0 + ns],
                          xkT_d[0:C, n0:n0 + ns].rearrange("(ch p) c -> p ch c", p=128))
    xks = [xks_tile[:, ci * NK:(ci + 1) * NK] for ci in range(4)]

    xs_tile = xpool.tile([128, 4 * T], BF16, name="xm", tag="xm")
    xs_v = xs_tile[:].rearrange("p (ch c) -> p ch c", ch=4)
    nc.gpsimd.dma_start(xs_v[:, :, 0:512],
                        xT_d[0:C, 0:512].rearrange("(ch p) c -> p ch c", p=128))
    wqs = load4(wpool, wq_d, 256, "wq", nc.gpsimd)
    for n0, ns in _chunks(T, 512)[1:]:
        nc.gpsimd.dma_start(xs_v[:, :, n0:n0 + ns],
                            xT_d[0:C, n0:n0 + ns].rearrange("(ch p) c -> p ch c", p=128))
    xs = [xs_tile[:, ci * T:(ci + 1) * T] for ci in range(4)]
    xk_aug = xpool.tile([1, NK], BF16, name="xka", tag="xka")
    nc.gpsimd.dma_start(xk_aug[:], xkT_d[C:C + 1, :])
    wv_aug = wpool.tile([1, 260], BF16, name="wva", tag="wva")
    nc.gpsimd.dma_start(wv_aug[:], wv_d[C:C + 1, :])
    cdrb = wpool.tile([128, max(NBT, 1)], F32, name="cdrb", tag="cdrb")
    nc.gpsimd.dma_start(cdrb[:], cdrb_d[:])
    wo_all = wpool.tile([128, 1024], BF16, name="wo", tag="wo")
    nc.gpsimd.dma_start(wo_all[:].rearrange("p (g c) -> p g c", g=2),
                        wo_d[:].rearrange("(g p) c -> p g c", p=128))
    wo = [wo_all[:, 0:512], wo_all[:, 512:1024]]

    # selector matrix for denominator broadcast: E.T @ cu = row 64 of cu
    # replicated across 64 psum partitions
    esel = wpool.tile([65, 64], BF16, name="esel", tag="esel")
    nc.vector.memset(esel[:], 0.0)
    nc.vector.memset(esel[64:65, :], 1.0)

    # ---- persistent activation tiles ------------------------------------
    qT = [qkv.tile([128, T], BF16, name=f"q{p}", tag=f"q{p}") for p in range(2)]
    kT = [qkv.tile([128, NK], BF16, name=f"k{p}", tag=f"k{p}") for p in range(2)]
    v_sb = qkv.tile([128, NKT * 260], BF16, name="v", tag="v")
    ctxn = [qkv.tile([128, T], BF16, name=f"ctxn{p}", tag=f"ctxn{p}")
            for p in range(2)]

    # ---- projection emitters --------------------------------------------
    def k_proj(pp):
        for n0, ns in _chunks(NK, 512):
            mt = psM.tile([128, 512], F32, name="m", tag="m")
            for ci in range(4):
                nc.tensor.matmul(
                    mt[:, :ns],
                    wks[ci][:, pp * 128:(pp + 1) * 128],
                    xks[ci][:, n0:n0 + ns],
                    start=(ci == 0), stop=(ci == 3))
            nc.vector.tensor_copy(kT[pp][:, n0:n0 + ns], mt[:, :ns])

    def v_proj(kt):
        mt = psM.tile([128, 512], F32, name="m", tag="m")
        for ci in range(4):
            nc.tensor.matmul(
                mt[:, 0:260],
                xks[ci][:, kt * 128:(kt + 1) * 128],
                wvs[ci][:],
                start=(ci == 0), stop=False)
        nc.tensor.matmul(
            mt[:, 0:260],
            xk_aug[0:1, kt * 128:(kt + 1) * 128],
            wv_aug[:],
            start=False, stop=True)
        nc.vector.tensor_copy(v_sb[:, kt * 260:(kt + 1) * 260], mt[:, 0:260])

    def q_proj(qc, pp):
        mt = psM.tile([128, 512], F32, name="m", tag="m")
        for ci in range(4):
            nc.tensor.matmul(
                mt[:],
                wqs[ci][:, pp * 128:(pp + 1) * 128],
                xs[ci][:, qc * 512:(qc + 1) * 512],
                start=(ci == 0), stop=(ci == 3))
        nc.vector.tensor_copy(qT[pp][:, qc * 512:(qc + 1) * 512], mt[:])

    def out_proj(qc, tqs):
        for tq in tqs:
            t0 = qc * 512 + tq * 128
            cp = psM.tile([128, 512], F32, name="m", tag="m")
            nc.tensor.matmul(cp[:], ctxn[0][:, t0:t0 + 128], wo[0],
                             start=True, stop=False)
            nc.tensor.matmul(cp[:], ctxn[1][:, t0:t0 + 128], wo[1],
                             start=False, stop=True)
            ot = pO.tile([128, 512], F32, name="ot", tag="o")
            nc.vector.tensor_copy(ot[:], cp[:])
            nc.sync.dma_start(out_d[t0:t0 + 128, :], ot[:])

    # ---- attention ------------------------------------------------------
    def attention(qc, pair):
        """Generator: yields once per ktile super-group so the caller can
        ration filler matmuls into the exp-wait bubbles."""
        q0 = qc * 512
        nts = [NKT_CDR if pair == 0 else NKT, NKT]
        Pt = pP.tile([128, NKT * 1024], BF16, name="P", tag="P")
        cp = [psC.tile([65, 512], F32, name=f"c{h}", tag="ctx")
              for h in range(2)]
        done = [0, 0]
        for kt in range(NKT):
            hs = [h for h in range(2) if kt < nts[h]]
            sp = psS.tile([128, 1024], F32, name="S", tag="S")
            # both heads' score matmuls for one ktile into one psum tile:
            # adjacent instructions on distinct PE row groups -> concurrent
            for h in hs:
                r0 = h * 64
                nc.tensor.matmul(
                    sp[:, h * 512:(h + 1) * 512],
                    kT[pair][r0:r0 + 64, kt * 128:(kt + 1) * 128],
                    qT[pair][r0:r0 + 64, q0:q0 + 512],
                    start=True, stop=True,
                    tile_position=(r0, 0))
            biased = (pair == 0 and 0 in hs and NBT > 0 and kt >= BIAS0)
            if len(hs) == 2 and not biased:
                nc.scalar.activation(
                    Pt[:, kt * 1024:(kt + 1) * 1024], sp[:],
                    Exp, scale=EXP_SCALE)
            else:
                for h in hs:
                    bias = (cdrb[:, kt - BIAS0:kt - BIAS0 + 1]
                            if (h == 0 and biased) else 0.0)
                    nc.scalar.activation(
                        Pt[:, (kt * 2 + h) * 512:(kt * 2 + h + 1) * 512],
                        sp[:, h * 512:(h + 1) * 512],
                        Exp, bias=bias, scale=EXP_SCALE)
            for h in hs:
                head = 2 * pair + h
                nc.tensor.matmul(
                    cp[h][:],
                    v_sb[:, kt * 260 + head * 65:kt * 260 + (head + 1) * 65],
                    Pt[:, (kt * 2 + h) * 512:(kt * 2 + h + 1) * 512],
                    start=(done[h] == 0), stop=(done[h] + 1 == nts[h]))
                done[h] += 1
                if done[h] == nts[h]:
                    # normalize: PE-broadcast the denominator row, then DVE
                    # fast-reciprocal + multiply at 64-partition width
                    r0 = h * 64
                    cu = pR.tile([65, 512], BF16, name="cu", tag="cu")
                    nc.vector.tensor_copy(cu[:], cp[h][:])
                    dn = psM.tile([128, 512], F32, name="m", tag="m")
                    nc.tensor.matmul(dn[0:64, :], esel[:], cu[:],
                                     start=True, stop=True)
                    rb = pR.tile([64, 512], F32, name="rb", tag="rb")
                    nc.vector.reciprocal_approx_fast(rb[:], dn[0:64, :])
                    nc.vector.tensor_mul(
                        ctxn[pair][r0:r0 + 64, q0:q0 + 512],
                        cu[0:64, :], rb[:])
                    done[h] += 1  # emit normalize once
            yield

    # ---- emission order -------------------------------------------------
    # PE warmup: dummy matmuls warm the HAM clock gate while input DMAs land
    wps = psM.tile([128, 512], F32, name="m", tag="m")
    for i in range(36):
        nc.tensor.matmul(wps[0:64, 0:64], esel[:], esel[:],
                         start=True, stop=True)
    nc.vector.tensor_copy(pR.tile([64, 64], F32, name="wd", tag="wd")[:],
                          wps[0:64, 0:64])

    k_proj(0)
    q_proj(0, 0)
    for kt in range(NKT):
        v_proj(kt)
    k_proj(1)
    q_proj(0, 1)
    for qc in range(4):
        for pair in range(2):
            fillers = []
            if qc > 0:
                tqs = (0, 1) if pair == 0 else (2, 3)
                fillers.append(lambda t=tqs[0]: out_proj(qc - 1, (t,)))
                fillers.append(lambda t=tqs[1]: out_proj(qc - 1, (t,)))
            if qc < 3:
                fillers.append(lambda p=pair: q_proj(qc + 1, p))
            fi = 0
            for _ in attention(qc, pair):
                if fi < len(fillers):
                    fillers[fi]()
                    fi += 1
            while fi < len(fillers):
                fillers[fi]()
                fi += 1
    out_proj(3, range(4))


# ---------------------------------------------------------------------------
# host side
# ---------------------------------------------------------------------------

def _host_prep(x, mask, cdrs_score, Wq, bq, Wk, bk, Wv, bv, Wo, bo):
    x = np.ascontiguousarray(np.asarray(x, np.float32))
    mask = np.asarray(mask)
    cdrs = np.asarray(cdrs_score)
    Wq = np.asarray(Wq, np.float32)
    Wk = np.asarray(Wk, np.float32)
    Wv = np.asarray(Wv, np.float32)
    Wo = np.asarray(Wo, np.float32)
    bv = np.asarray(bv, np.float32)
    assert np.abs(np.asarray(bq)).max() < 1e-6, "nonzero bq unsupported"
    assert np.abs(np.asarray(bk)).max() < 1e-6, "nonzero bk unsupported"

    gathers = []
    for b in range(B):
        valid = mask[b] == 1
        cdrv = valid & (cdrs[b] == 1) if np.any(cdrs[b] == 1) else valid
        regv = valid & ~cdrv
        gathers.append((np.nonzero(cdrv)[0], np.nonzero(regv)[0]))
    ncdrs = [len(g[0]) for g in gathers]
    valids = [len(g[0]) + len(g[1]) for g in gathers]
    NKT = max(1, math.ceil(max(valids) / 128))
    NK = NKT * 128
    NKT_CDR = max(1, math.ceil(max(ncdrs) / 128))
    BIAS0 = min(ncdrs) // 128
    NBT = NKT_CDR - BIAS0

    # per-group weight bundles (shared across samples)
    wbund = []
    for g in range(2):
        heads = [g, g + 2, g + 4, g + 6]
        dims = np.concatenate([np.arange(h * D, (h + 1) * D) for h in heads])
        wq_c = Wq[:, dims]
        wk_c = Wk[:, dims]
        wv_cols = []
        for h in heads:
            hd = np.arange(h * D, (h + 1) * D)
            wv = np.concatenate([Wv[:, hd], bv[hd][None, :]], axis=0)
            sel = np.zeros((C + 1, 1), np.float32)
            sel[C, 0] = 1.0
            wv_cols.append(np.concatenate([wv, sel], axis=1))
        wv_aug = np.concatenate(wv_cols, axis=1)
        wo_rows = Wo[dims, :]
        wbund.append(tuple(
            np.ascontiguousarray(w.astype(ml_dtypes.bfloat16))
            for w in (wq_c, wk_c, wv_aug, wo_rows)))

    in_maps = []
    for b in range(B):
        idx_cdr, idx_reg = gathers[b]
        nv = len(idx_cdr) + len(idx_reg)
        xk = np.zeros((NK, C), np.float32)
        xk[:len(idx_cdr)] = x[b, idx_cdr]
        xk[len(idx_cdr):nv] = x[b, idx_reg]
        ones_row = np.zeros((1, NK), np.float32)
        ones_row[0, :nv] = 1.0
        xkT_aug = np.ascontiguousarray(
            np.concatenate([xk.T, ones_row], axis=0))
        xT_bf = np.ascontiguousarray(x[b].T.astype(ml_dtypes.bfloat16))
        xkT_bf = np.ascontiguousarray(xkT_aug.astype(ml_dtypes.bfloat16))
        cdrb = np.zeros((128, max(NBT, 1)), np.float32)
        for t in range(NBT):
            keys = (BIAS0 + t) * 128 + np.arange(128)
            cdrb[:, t] = np.where(keys < len(idx_cdr), 0.0, MASK_BIAS)
        for g in range(2):
            wq_c, wk_c, wv_aug, wo_rows = wbund[g]
            in_maps.append({
                "xT": xT_bf, "xkT": xkT_bf,
                "Wq": wq_c, "Wk": wk_c, "Wv": wv_aug, "Wo": wo_rows,
                "cdrb": cdrb,
            })
    return in_maps, NKT, NKT_CDR, BIAS0, NBT


def kernel(**inputs) -> np.ndarray:
    global LAST_RESULTS
    in_maps, NKT, NKT_CDR, BIAS0, NBT = _host_prep(**inputs)

    key = (NKT, NKT_CDR, BIAS0, NBT)
    nc = _PROGRAM_CACHE.get(key)
    if nc is None:
        nc = _build_program(NKT, NKT_CDR, BIAS0, NBT)
        _PROGRAM_CACHE[key] = nc

    res = run_bass_kernel_spmd(nc, in_maps, core_ids=list(range(8)))
    LAST_RESULTS = res

    bo = np.asarray(inputs["bo"], np.float32)
    out = np.empty((B, T, C), np.float32)
    for b in range(B):
        out[b] = res.results[2 * b]["out"] + res.results[2 * b + 1]["out"] + bo[None, :]
    return out


# revision 16
# speedup vs baseline: 1.1743x; 1.1463x over previous
"""Trainium2 Bass kernel for nn_CDRsAttention (sparse multi-head attention
with padding mask + CDR key mask on the first 2 heads).

Sharding: 8 cores = 4 samples (B) x 2 head groups. Core (b, g) computes
heads [g, g+2, g+4, g+6] of sample b (exactly one CDR head each), producing
a partial output ctx_heads @ Wo_rows; the host sums the two partials + bo.

Host-side prep (pure numpy, cheap):
  - per-sample key gather: only keys with mask==1 participate, CDR-valid
    keys first, then regular keys, zero-padded to NK = 128*ceil(max valid).
    No inter-region padding: the CDR head attends tiles [0, NKT_CDR) and
    masks intruding regular keys via a per-partition additive bias (-30)
    fed to the exp activation for the boundary tiles.
  - xkT carries one extra row (valid-key indicator) that flows through
    augmented Wv selector columns so v column h*65+64 is the indicator row,
    making ctx^T row 64 the softmax denominator (padded keys drop out).
  - q/k biases are zero, so their projections contract over exactly 512
    rows (no augmented row).

Device per core (bf16 matmuls):
  qT/kT/v projections -> per head: S^T = kT_tile^T @ qT (keys on psum
  partitions), P = exp(S^T/8) on ScalarE straight out of PSUM (pairs of
  ktiles per activate), ctx^T accumulated as v_aug^T @ P, then a
  fast-reciprocal of the denominator row, a PE broadcast matmul (f32r),
  a DVE normalization multiply, and the output projection
  out = ctx_norm^T.T @ Wo_rows streamed straight to DRAM per 128-query
  slab. q/out projections are interleaved as PE filler between attention
  groups to keep the PE array dense and HAM-warm.
"""
import math
from contextlib import ExitStack

import ml_dtypes
import numpy as np

import concourse.bass as bass
import concourse.mybir as mybir
import concourse.tile as tile
from concourse import bacc
from concourse.bass_utils import run_bass_kernel_spmd

B, T, C, H, D = 4, 2048, 512, 8, 64
F32 = mybir.dt.float32
F32R = mybir.dt.float32r
BF16 = mybir.dt.bfloat16
EXP_SCALE = 1.0 / 8.0  # 1/sqrt(D)
MASK_BIAS = -30.0

_PROGRAM_CACHE: dict = {}
LAST_RESULTS = None  # BassKernelResults of the most recent kernel() call


def _chunks(total, step):
    return [(i, min(step, total - i)) for i in range(0, total, step)]


def _build_program(NKT, NKT_CDR, BIAS0, NBT):
    NK = NKT * 128
    nc = bacc.Bacc("TRN2", target_bir_lowering=False, debug=False, num_devices=8)
    xT_d = nc.dram_tensor("xT", [C, T], BF16, kind="ExternalInput").ap()
    xkT_d = nc.dram_tensor("xkT", [C + 1, NK], BF16, kind="ExternalInput").ap()
    wq_d = nc.dram_tensor("Wq", [C, 256], BF16, kind="ExternalInput").ap()
    wk_d = nc.dram_tensor("Wk", [C, 256], BF16, kind="ExternalInput").ap()
    wv_d = nc.dram_tensor("Wv", [C + 1, 260], BF16, kind="ExternalInput").ap()
    wo_d = nc.dram_tensor("Wo", [256, 512], BF16, kind="ExternalInput").ap()
    cdrb_d = nc.dram_tensor("cdrb", [128, max(NBT, 1)], F32,
                            kind="ExternalInput").ap()
    out_d = nc.dram_tensor("out", [T, 512], F32, kind="ExternalOutput").ap()

    with tile.TileContext(nc) as tc:
        with ExitStack() as ctx:
            _body(ctx, tc, xT_d, xkT_d, wq_d, wk_d, wv_d, wo_d, cdrb_d, out_d,
                  NK, NKT, NKT_CDR, BIAS0, NBT)
    nc.compile()
    return nc


def _body(ctx, tc, xT_d, xkT_d, wq_d, wk_d, wv_d, wo_d, cdrb_d, out_d,
          NK, NKT, NKT_CDR, BIAS0, NBT):
    nc = tc.nc
    Exp = mybir.ActivationFunctionType.Exp

    wpool = ctx.enter_context(tc.tile_pool(name="w", bufs=1))
    xpool = ctx.enter_context(tc.tile_pool(name="x", bufs=1))
    qkv = ctx.enter_context(tc.tile_pool(name="qkv", bufs=1))
    psS = ctx.enter_context(tc.tile_pool(name="psS", bufs=2, space="PSUM"))
    psC = ctx.enter_context(tc.tile_pool(name="psC", bufs=2, space="PSUM"))
    psM = ctx.enter_context(tc.tile_pool(name="psM", bufs=2, space="PSUM"))
    pP = ctx.enter_context(tc.tile_pool(name="pP", bufs=2))
    pR = ctx.enter_context(tc.tile_pool(name="pR", bufs=2))
    pO = ctx.enter_context(tc.tile_pool(name="pO", bufs=3))

    # ---- input loads: sync queue feeds k/v path, gpsimd queue the rest ----
    def load4(pool, dram, cols, nm, eng, col_chunks=None):
        """[512, cols] DRAM -> [128, 4*cols] tile; 4 contraction-chunk views."""
        main = pool.tile([128, 4 * cols], BF16, name=f"{nm}m", tag=f"{nm}m")
        mv = main[:].rearrange("p (ch c) -> p ch c", ch=4)
        for n0, ns in (col_chunks or [(0, cols)]):
            eng.dma_start(
                mv[:, :, n0:n0 + ns],
                dram[0:C, n0:n0 + ns].rearrange("(ch p) c -> p ch c", p=128))
        return [main[:, ci * cols:(ci + 1) * cols] for ci in range(4)]

    wks = load4(wpool, wk_d, 256, "wk", nc.sync)
    nkch = _chunks(NK, 512)
    xks_tile = xpool.tile([128, 4 * NK], BF16, name="xkm", tag="xkm")
    xks_v = xks_tile[:].rearrange("p (ch c) -> p ch c", ch=4)
    n0, ns = nkch[0]
    nc.sync.dma_start(xks_v[:, :, n0:n0 + ns],
                      xkT_d[0:C, n0:n0 + ns].rearrange("(ch p) c -> p ch c", p=128))
    wvs = load4(wpool, wv_d, 260, "wv", nc.sync)
    for n0, ns in nkch[1:]:
        nc.sync.dma_start(xks_v[:, :, n0:n0 + ns],
                          xkT_d[0:C, n0:n0 + ns].rearrange("(ch p) c -> p ch c", p=128))
    xks = [xks_tile[:, ci * NK:(ci + 1) * NK] for ci in range(4)]

    # q-path on the gpsimd queue: first x chunk + Wq, then small tensors;
    # the bulky remaining x chunks and Wo are deferred behind the k-path
    xs_tile = xpool.tile([128, 4 * T], BF16, name="xm", tag="xm")
    xs_v = xs_tile[:].rearrange("p (ch c) -> p ch c", ch=4)

    def x_chunk(eng, n0, ns):
        eng.dma_start(xs_v[:, :, n0:n0 + ns],
                      xT_d[0:C, n0:n0 + ns].rearrange("(ch p) c -> p ch c", p=128))

    x_chunk(nc.gpsimd, 0, 512)
    wqs = load4(wpool, wq_d, 256, "wq", nc.gpsimd)
    xs = [xs_tile[:, ci * T:(ci + 1) * T] for ci in range(4)]
    xk_aug = xpool.tile([1, NK], BF16, name="xka", tag="xka")
    nc.gpsimd.dma_start(xk_aug[:], xkT_d[C:C + 1, :])
    wv_aug = wpool.tile([1, 260], BF16, name="wva", tag="wva")
    nc.gpsimd.dma_start(wv_aug[:], wv_d[C:C + 1, :])
    cdrb = wpool.tile([128, max(NBT, 1)], F32, name="cdrb", tag="cdrb")
    nc.gpsimd.dma_start(cdrb[:], cdrb_d[:])
    x_chunk(nc.gpsimd, 512, 512)
    x_chunk(nc.sync, 1024, 512)
    wo_all = wpool.tile([128, 1024], BF16, name="wo", tag="wo")
    nc.gpsimd.dma_start(wo_all[:].rearrange("p (g c) -> p g c", g=2),
                        wo_d[:].rearrange("(g p) c -> p g c", p=128))
    wo = [wo_all[:, 0:512], wo_all[:, 512:1024]]
    x_chunk(nc.sync, 1536, 512)

    # selector matrix for denominator broadcast: E.T @ cu = row 64 of cu
    # replicated across 64 psum partitions
    esel = wpool.tile([65, 64], BF16, name="esel", tag="esel")
    nc.vector.memset(esel[:], 0.0)
    nc.vector.memset(esel[64:65, :], 1.0)

    # ---- persistent activation tiles ------------------------------------
    qT = [qkv.tile([128, T], BF16, name=f"q{p}", tag=f"q{p}") for p in range(2)]
    kT = [qkv.tile([128, NK], BF16, name=f"k{p}", tag=f"k{p}") for p in range(2)]
    v_sb = qkv.tile([128, NKT * 260], BF16, name="v", tag="v")
    ctxn = [qkv.tile([128, T], BF16, name=f"ctxn{p}", tag=f"ctxn{p}")
            for p in range(2)]

    # ---- projection emitters --------------------------------------------
    def k_proj(pp):
        for n0, ns in _chunks(NK, 512):
            mt = psM.tile([128, 512], F32, name="m", tag="m")
            for ci in range(4):
                nc.tensor.matmul(
                    mt[:, :ns],
                    wks[ci][:, pp * 128:(pp + 1) * 128],
                    xks[ci][:, n0:n0 + ns],
                    start=(ci == 0), stop=(ci == 3))
            nc.vector.tensor_copy(kT[pp][:, n0:n0 + ns], mt[:, :ns])

    def v_proj(kt):
        mt = psM.tile([128, 512], F32, name="m", tag="m")
        for ci in range(4):
            nc.tensor.matmul(
                mt[:, 0:260],
                xks[ci][:, kt * 128:(kt + 1) * 128],
                wvs[ci][:],
                start=(ci == 0), stop=False)
        nc.tensor.matmul(
            mt[:, 0:260],
            xk_aug[0:1, kt * 128:(kt + 1) * 128],
            wv_aug[:],
            start=False, stop=True)
        nc.vector.tensor_copy(v_sb[:, kt * 260:(kt + 1) * 260], mt[:, 0:260])

    def q_proj(qc, pp):
        mt = psM.tile([128, 512], F32, name="m", tag="m")
        for ci in range(4):
            nc.tensor.matmul(
                mt[:],
                wqs[ci][:, pp * 128:(pp + 1) * 128],
                xs[ci][:, qc * 512:(qc + 1) * 512],
                start=(ci == 0), stop=(ci == 3))
        nc.vector.tensor_copy(qT[pp][:, qc * 512:(qc + 1) * 512], mt[:])

    def out_proj(qc, tqs):
        for tq in tqs:
            t0 = qc * 512 + tq * 128
            cp = psM.tile([128, 512], F32, name="m", tag="m")
            nc.tensor.matmul(cp[:], ctxn[0][:, t0:t0 + 128], wo[0],
                             start=True, stop=False)
            nc.tensor.matmul(cp[:], ctxn[1][:, t0:t0 + 128], wo[1],
                             start=False, stop=True)
            ot = pO.tile([128, 512], F32, name="ot", tag="o")
            nc.vector.tensor_copy(ot[:], cp[:])
            nc.sync.dma_start(out_d[t0:t0 + 128, :], ot[:])

    # ---- attention ------------------------------------------------------
    # Split into a scores+exp pass (A) and a ctx+normalize pass (B). A(i)
    # and B(i-1) are emitted interleaved: while phase i's exp activates
    # pace the scalar engine, the previous phase's ctx matmuls (dep-free,
    # P is already in SBUF) keep the PE array dense.
    def attn_scores(qc, pair):
        Pt = pP.tile([128, NKT * 1024], BF16, name="P", tag="P")
        return Pt, _attn_scores_gen(qc, pair, Pt)

    def _attn_scores_gen(qc, pair, Pt):
        q0 = qc * 512
        nts = [NKT_CDR if pair == 0 else NKT, NKT]
        for kt in range(NKT):
            hs = [h for h in range(2) if kt < nts[h]]
            sp = psS.tile([128, 1024], F32, name="S", tag="S")
            # both heads' score matmuls for one ktile into one psum tile:
            # adjacent instructions on distinct PE row groups -> concurrent
            for h in hs:
                r0 = h * 64
                nc.tensor.matmul(
                    sp[:, h * 512:(h + 1) * 512],
                    kT[pair][r0:r0 + 64, kt * 128:(kt + 1) * 128],
                    qT[pair][r0:r0 + 64, q0:q0 + 512],
                    start=True, stop=True,
                    tile_position=(r0, 0))
            biased = (pair == 0 and 0 in hs and NBT > 0 and kt >= BIAS0)
            if len(hs) == 2 and not biased:
                nc.scalar.activation(
                    Pt[:, kt * 1024:(kt + 1) * 1024], sp[:],
                    Exp, scale=EXP_SCALE)
            else:
                for h in hs:
                    bias = (cdrb[:, kt - BIAS0:kt - BIAS0 + 1]
                            if (h == 0 and biased) else 0.0)
                    nc.scalar.activation(
                        Pt[:, (kt * 2 + h) * 512:(kt * 2 + h + 1) * 512],
                        sp[:, h * 512:(h + 1) * 512],
                        Exp, bias=bias, scale=EXP_SCALE)
            yield
        return

    def attn_ctx(qc, pair, Pt):
        q0 = qc * 512
        nts = [NKT_CDR if pair == 0 else NKT, NKT]
        cp = [psC.tile([65, 512], F32, name=f"c{h}", tag="ctx")
              for h in range(2)]
        done = [0, 0]
        for kt in range(NKT):
            hs = [h for h in range(2) if kt < nts[h]]
            for h in hs:
                head = 2 * pair + h
                nc.tensor.matmul(
                    cp[h][:],
                    v_sb[:, kt * 260 + head * 65:kt * 260 + (head + 1) * 65],
                    Pt[:, (kt * 2 + h) * 512:(kt * 2 + h + 1) * 512],
                    start=(done[h] == 0), stop=(done[h] + 1 == nts[h]))
                done[h] += 1
                if done[h] == nts[h]:
                    # normalize: PE-broadcast the denominator row, then DVE
                    # fast-reciprocal + multiply at 64-partition width
                    r0 = h * 64
                    cu = pR.tile([65, 512], BF16, name="cu", tag="cu")
                    nc.vector.tensor_copy(cu[:], cp[h][:])
                    dn = psM.tile([128, 512], F32, name="m", tag="m")
                    nc.tensor.matmul(dn[0:64, :], esel[:], cu[:],
                                     start=True, stop=True)
                    rb = pR.tile([64, 512], F32, name="rb", tag="rb")
                    nc.vector.reciprocal_approx_fast(rb[:], dn[0:64, :])
                    nc.vector.tensor_mul(
                        ctxn[pair][r0:r0 + 64, q0:q0 + 512],
                        cu[0:64, :], rb[:])
            yield

    # ---- emission order -------------------------------------------------
    # PE warmup: dummy matmuls warm the HAM clock gate while input DMAs land
    wps = psM.tile([128, 512], F32, name="m", tag="m")
    for i in range(80):
        nc.tensor.matmul(wps[0:64, 0:64], esel[:], esel[:],
                         start=True, stop=True)
    nc.vector.tensor_copy(pR.tile([64, 64], F32, name="wd", tag="wd")[:],
                          wps[0:64, 0:64])

    k_proj(0)
    q_proj(0, 0)
    for kt in range(NKT):
        v_proj(kt)
    k_proj(1)
    q_proj(0, 1)

    def exhaust(gen):
        if gen is not None:
            for _ in gen:
                pass

    phases = [(qc, pair) for qc in range(4) for pair in range(2)]
    prevPt = None
    for i, (qc, pair) in enumerate(phases):
        Pt, A = attn_scores(qc, pair)
        B = attn_ctx(*phases[i - 1], prevPt) if i > 0 else None
        fillers = []
        if i + 1 < len(phases) and phases[i + 1][0] >= 1:
            nqc, npair = phases[i + 1]
            fillers.append(lambda q=nqc, p=npair: q_proj(q, p))
        if i >= 3:
            qd = (i - 3) // 2
            tqs = (0, 1) if (i - 3) % 2 == 0 else (2, 3)
            fillers.append(lambda q=qd, t=tqs[0]: out_proj(q, (t,)))
            fillers.append(lambda q=qd, t=tqs[1]: out_proj(q, (t,)))
        fi = 0
        step = 0
        for _ in A:
            if B is not None:
                try:
                    next(B)
                except StopIteration:
                    B = None
            if step % 3 == 2 and fi < len(fillers):
                fillers[fi]()
                fi += 1
            step += 1
        exhaust(B)
        while fi < len(fillers):
            fillers[fi]()
            fi += 1
        prevPt = Pt
    exhaust(attn_ctx(*phases[-1], prevPt))
    out_proj(2, (2, 3))
    out_proj(3, range(4))


# ---------------------------------------------------------------------------
# host side
# ---------------------------------------------------------------------------

def _host_prep(x, mask, cdrs_score, Wq, bq, Wk, bk, Wv, bv, Wo, bo):
    x = np.ascontiguousarray(np.asarray(x, np.float32))
    mask = np.asarray(mask)
    cdrs = np.asarray(cdrs_score)
    Wq = np.asarray(Wq, np.float32)
    Wk = np.asarray(Wk, np.float32)
    Wv = np.asarray(Wv, np.float32)
    Wo = np.asarray(Wo, np.float32)
    bv = np.asarray(bv, np.float32)
    assert np.abs(np.asarray(bq)).max() < 1e-6, "nonzero bq unsupported"
    assert np.abs(np.asarray(bk)).max() < 1e-6, "nonzero bk unsupported"

    gathers = []
    for b in range(B):
        valid = mask[b] == 1
        cdrv = valid & (cdrs[b] == 1) if np.any(cdrs[b] == 1) else valid
        regv = valid & ~cdrv
        gathers.append((np.nonzero(cdrv)[0], np.nonzero(regv)[0]))
    ncdrs = [len(g[0]) for g in gathers]
    valids = [len(g[0]) + len(g[1]) for g in gathers]
    NKT = max(1, math.ceil(max(valids) / 128))
    NK = NKT * 128
    NKT_CDR = max(1, math.ceil(max(ncdrs) / 128))
    BIAS0 = min(ncdrs) // 128
    NBT = NKT_CDR - BIAS0

    # per-group weight bundles (shared across samples)
    wbund = []
    for g in range(2):
        heads = [g, g + 2, g + 4, g + 6]
        dims = np.concatenate([np.arange(h * D, (h + 1) * D) for h in heads])
        wq_c = Wq[:, dims]
        wk_c = Wk[:, dims]
        wv_cols = []
        for h in heads:
            hd = np.arange(h * D, (h + 1) * D)
            wv = np.concatenate([Wv[:, hd], bv[hd][None, :]], axis=0)
            sel = np.zeros((C + 1, 1), np.float32)
            sel[C, 0] = 1.0
            wv_cols.append(np.concatenate([wv, sel], axis=1))
        wv_aug = np.concatenate(wv_cols, axis=1)
        wo_rows = Wo[dims, :]
        wbund.append(tuple(
            np.ascontiguousarray(w.astype(ml_dtypes.bfloat16))
            for w in (wq_c, wk_c, wv_aug, wo_rows)))

    in_maps = []
    for b in range(B):
        idx_cdr, idx_reg = gathers[b]
        nv = len(idx_cdr) + len(idx_reg)
        xk = np.zeros((NK, C), np.float32)
        xk[:len(idx_cdr)] = x[b, idx_cdr]
        xk[len(idx_cdr):nv] = x[b, idx_reg]
        ones_row = np.zeros((1, NK), np.float32)
        ones_row[0, :nv] = 1.0
        xkT_aug = np.ascontiguousarray(
            np.concatenate([xk.T, ones_row], axis=0))
        xT_bf = np.ascontiguousarray(x[b].T.astype(ml_dtypes.bfloat16))
        xkT_bf = np.ascontiguousarray(xkT_aug.astype(ml_dtypes.bfloat16))
        cdrb = np.zeros((128, max(NBT, 1)), np.float32)
        for t in range(NBT):
            keys = (BIAS0 + t) * 128 + np.arange(128)
            cdrb[:, t] = np.where(keys < len(idx_cdr), 0.0, MASK_BIAS)
        for g in range(2):
            wq_c, wk_c, wv_aug, wo_rows = wbund[g]
            in_maps.append({
                "xT": xT_bf, "xkT": xkT_bf,
                "Wq": wq_c, "Wk": wk_c, "Wv": wv_aug, "Wo": wo_rows,
                "cdrb": cdrb,
            })
    return in_maps, NKT, NKT_CDR, BIAS0, NBT


def kernel(**inputs) -> np.ndarray:
    global LAST_RESULTS
    in_maps, NKT, NKT_CDR, BIAS0, NBT = _host_prep(**inputs)

    key = (NKT, NKT_CDR, BIAS0, NBT)
    nc = _PROGRAM_CACHE.get(key)
    if nc is None:
        nc = _build_program(NKT, NKT_CDR, BIAS0, NBT)
        _PROGRAM_CACHE[key] = nc

    res = run_bass_kernel_spmd(nc, in_maps, core_ids=list(range(8)))
    LAST_RESULTS = res

    bo = np.asarray(inputs["bo"], np.float32)
    out = np.empty((B, T, C), np.float32)
    for b in range(B):
        out[b] = res.results[2 * b]["out"] + res.results[2 * b + 1]["out"] + bo[None, :]
    return out
